# revision 31
# baseline (speedup 1.0000x reference)
"""AttentionConv (sparse local attention, 7x7 window, per-channel softmax)
Trainium2 Bass kernel, SPMD across 8 NeuronCores.

Sharding: core i handles batch b = i//2 and channel half cg = i%2
(channels are independent through the whole op: 1x1 convs produce each
output channel from all input channels, and the softmax is per-channel
over the 7x7 window).

The relative-position bias for channels [0,128) is rel_h[u] (window row)
and for channels [128,256) is rel_w[v] (window col). To keep one SPMD
program for all cores, cg=1 cores receive spatially TRANSPOSED x (H<->W)
and their output is transposed back on the host; under that transpose
rel_w becomes a window-row bias, identical in structure to cg=0.

Per-core pipeline (fp16 score path, bf16 value path, f32 accumulate):
  1. PE GEMMs: q,k,v = W @ x in fp16 (negligible rounding vs fp32 for
     this data, half the DMA bytes, 16-bit matmul speed). K=256
     contraction in 2 chunks, N chunks of 512 (one PSUM bank each).
  2. k,v scattered into zero-padded 38x40 planes; each plane stored
     twice (interior at col 3 and col 2) so windowed reads for even AND
     odd window-cols are 4-byte aligned -> DVE 16-bit 2x perf mode.
  3. 7 bias-added copies of each padded k plane (bias for a fixed
     window-row is a per-partition scalar -> tensor_scalar 4x mode).
  4. main loop over window col v (7 iters), u-dim split in (4,3) halves
     for pipelining, diagonal access patterns covering all u at once:
       s = q * k_biased[window]  fp16     (DVE TT, 2x)
       e = exp(s) -> bf16                 (ScalarE ACT, unnormalized --
                                           scores are far inside exp's
                                           f32/bf16 range, so no
                                           max-subtraction pass needed)
       m = e * v[window]  bf16            (DVE TT, 2x)
       num += I @ m ; den += I @ e        (TensorE identity matmuls
                                           accumulating in PSUM f32;
                                           the otherwise-idle PE does
                                           all the j-summation work)
  5. out = num * reciprocal_approx_fast(den); split-queue DMA out.
Engine budget per core: DVE ~62us (bottleneck: 2 multiplies per window
element at 2 elem/cyc/lane), ACT ~46us, PE ~52us, ~85us measured total.
"""

import os

import numpy as np
import ml_dtypes

K = 7
PAD = 3
H = W = 32
HW = H * W
B = 4
C = 256
RS = 40          # padded plane row stride (elements); even => alignment
PR = H + 2 * PAD  # 38 padded rows
PW = PR * RS     # padded plane size per partition
N_CORES = 8

_NC_CACHE = {}


def _build_nc():
    import concourse.bass as bass
    import concourse.tile as tile
    from concourse import mybir, bacc

    bf16 = mybir.dt.bfloat16
    f16 = mybir.dt.float16
    f32 = mybir.dt.float32

    nc = bacc.Bacc(None)
    x_ext = nc.dram_tensor("x", [128, 2, HW], f16, kind="ExternalInput")
    w_ext = nc.dram_tensor("w", [3, 128, 2, 128], f16, kind="ExternalInput")
    b_ext = nc.dram_tensor("bias", [128, K], f32, kind="ExternalInput")
    i_ext = nc.dram_tensor("ident", [128, 128], bf16, kind="ExternalInput")
    o_ext = nc.dram_tensor("out", [128, HW], f32, kind="ExternalOutput")

    with tile.TileContext(nc) as tc:
        with (
            tc.tile_pool(name="consts", bufs=1) as consts,
            tc.tile_pool(name="kv", bufs=1) as kv,
            tc.tile_pool(name="fin", bufs=1) as fin,
            tc.tile_pool(name="psa", bufs=1, space="PSUM") as psa,
            tc.tile_pool(name="gt", bufs=1) as gt,
            tc.tile_pool(name="psg", bufs=4, space="PSUM") as psg,
            tc.tile_pool(name="sp", bufs=3) as sp,
            tc.tile_pool(name="ep", bufs=3) as ep,
            tc.tile_pool(name="mp", bufs=3) as mp,
        ):
            # Batched DMAs: each trigger instruction costs ~650ns of queue
            # time, so few big transfers beat many small ones. k weights
            # and the first pixel-half of x go first (they gate the k GEMM
            # -> scatter -> bias-copy -> main-loop chain).
            xsb = gt.tile([128, 2, HW], f16)
            wsb = gt.tile([128, 3, 2, 128], f16)
            bsb = consts.tile([128, K], f32)
            isb = consts.tile([128, 128], bf16)
            # (GpSimd stays trigger-free: its queue must reach the plane
            # memsets ASAP -- they gate the k scatter. ident goes late:
            # the warm-up matmuls chew on wk instead, and the first real
            # identity matmul only fires ~14us in.)
            nc.sync.dma_start(out=wsb[:, 1], in_=w_ext[1])
            nc.scalar.dma_start(out=xsb[:, :, 0:512], in_=x_ext[:, :, 0:512])
            nc.sync.dma_start(out=xsb[:, :, 512:HW], in_=x_ext[:, :, 512:HW])
            nc.scalar.dma_start(out=wsb[:, 0], in_=w_ext[0])
            nc.sync.dma_start(out=isb[:], in_=i_ext[:])
            nc.scalar.dma_start(out=wsb[:, 2], in_=w_ext[2])
            nc.sync.dma_start(out=bsb[:], in_=b_ext[:])

            # q plane (read via a 7-way broadcast AP in the main loop)
            qsb = kv.tile([128, H, W], f16)
            # padded k/v planes; E holds interior at col 3 (for even v
            # window reads), O at col 2 (odd v reads at offset v-1).
            kpE = gt.tile([128, PR, RS], f16)
            kpO = gt.tile([128, PR, RS], f16)
            vpE = kv.tile([128, PR, RS], bf16)
            vpO = kv.tile([128, PR, RS], bf16)
            # kb planes are stored pre-shifted: kb[u][r] = kp[u+r] + b[u]
            # for r in 0..32 (only rows u..u+31 of a padded plane are
            # ever read for window-row u). Copies shrink 1520->1280
            # elems and the window AP's u-step becomes exactly one
            # plane (PW32).
            PW32 = H * RS
            kbE = kv.tile([128, K, H, RS], f16)
            kbO = kv.tile([128, K, H, RS], f16)

            nc.gpsimd.memset(kpE[:], 0.0)
            nc.gpsimd.memset(kpO[:], 0.0)
            nc.gpsimd.memset(vpE[:], 0.0)
            nc.gpsimd.memset(vpO[:], 0.0)

            # per-chunk PSUM accumulators (finer deps => earlier tail)
            nps0 = psa.tile([128, 512], f32)
            nps1 = psa.tile([128, 512], f32)
            dps0 = psa.tile([128, 512], f32)
            dps1 = psa.tile([128, 512], f32)
            nps = [nps0, nps1]
            dps = [dps0, dps1]

            # PE pipeline/HAM warm-up: dummy matmuls into nps0, whose
            # content is discarded when the first start=True accumulation
            # clears has_written. They chew on the k weights (first DMA
            # to land) so they don't wait for the ident transfer.
            for _ in range(3):
                nc.tensor.matmul(nps0[:, 0:128], wsb[:, 1, 0, :],
                                 wsb[:, 1, 0, :],
                                 start=True, stop=True, skip_group_check=True)

            # ---- GEMMs: wi 0=q, 1=k, 2=v; N chunks of 512 px (16 rows)
            # k+q matmuls first (4 PSUM bufs hold k0,k1,q0,q1), with the
            # ACT scatter order kpE -> q -> kpO so the chain that gates
            # the first main-loop TT (kpE scatter -> kbE copies on DVE)
            # is as early as possible. v GEMMs reuse the k PSUM bufs
            # after the kpO scatter releases them.
            def gemm(wi, ch):
                ps = psg.tile([128, 16, 32], f32, tag="ps", name="ps")
                for ci in range(2):
                    nc.tensor.matmul(
                        ps[:],
                        wsb[:, wi, ci, :],
                        xsb[:, ci, ch * 512:(ch + 1) * 512],
                        start=(ci == 0),
                        stop=(ci == 1),
                    )
                return ps

            ps_k = [gemm(1, ch) for ch in range(2)]
            ps_q = [gemm(0, ch) for ch in range(2)]
            # biased k copies: kb*[u] = kp* + bias[:, u] (per-partition).
            # All on DVE (4x tensor_scalar mode; GpSimd's tensor_scalar
            # ucode measures ~22us/plane regardless of AP shape, and ACT
            # has no slack once the exp stream starts). The gate for the
            # first TT -- kbE[0..3] -- is split by plane row-halves so
            # the ch0-row copies overlap the k-ch1 GEMM+scatter;
            # kbE[4..6] and all of kbO are emitted inside v=0's body
            # where they fill DVE's wait for the first exp.
            nc.scalar.copy(kpE[:, PAD:PAD + 16, 3:35], ps_k[0][:])
            for u in range(4):
                b0 = PAD + 16 - u  # shifted-plane row where ch1 data starts
                nc.vector.tensor_scalar_add(
                    kbE[:, u, 0:b0], kpE[:, u:u + b0], bsb[:, u:u + 1])
            nc.scalar.copy(kpE[:, PAD + 16:PAD + 32, 3:35], ps_k[1][:])
            for ch in range(2):
                nc.scalar.copy(qsb[:, 16 * ch:16 * ch + 16, :], ps_q[ch][:])
            for u in range(4):
                b0 = PAD + 16 - u
                nc.vector.tensor_scalar_add(
                    kbE[:, u, b0:H], kpE[:, u + b0:u + H],
                    bsb[:, u:u + 1])
            for ch in range(2):
                r0 = PAD + 16 * ch
                nc.scalar.copy(kpO[:, r0:r0 + 16, 2:34], ps_k[ch][:])
            for ch in range(2):
                r0 = PAD + 16 * ch
                ps = gemm(2, ch)
                nc.scalar.copy(vpE[:, r0:r0 + 16, 3:35], ps[:])
                nc.scalar.copy(vpO[:, r0:r0 + 16, 2:34], ps[:])

            def window_ap(t, base_off, u0, nu, u_step):
                full = t[:]
                return bass.AP(
                    tensor=full.tensor,
                    offset=full.offset + base_off + u0 * u_step,
                    ap=[full.ap[0], [u_step, nu], [RS, H], [1, W]],
                )

            def q_bcast(nu):
                full = qsb[:]
                return bass.AP(
                    tensor=full.tensor,
                    offset=full.offset,
                    ap=[full.ap[0], [0, nu], [W, H], [1, W]],
                )

            # u-dim halves for finer DVE->ACT->PE pipelining. Measured
            # optimum: full-width ops 89.8us, quarters 87.2us, (3,4)
            # order 88.2us, this (4,3) split 85.4us. GpSimd offloads of
            # any slab regress (its in-loop TT is 3-5x slower than DVE).
            HALVES = ((0, 4), (4, 3))

            # ---- main loop over window col v (all elementwise on DVE;
            # GpSimd in-loop offload measured 4.2ns/elem under SBUF
            # contention + 1.7us drains and cannot hold the ~6.5us/v
            # cadence -- it lagged the whole loop to 133us)
            for v in range(K):
                par = v & 1
                kb = kbO if par else kbE
                vp = vpO if par else vpE
                off = v - par  # even

                first = v == 0
                last = v == K - 1
                mm_args = []
                for u0, nu in HALVES:
                    s = sp.tile([128, nu, H, W], f16, tag=f"s{u0}")
                    nc.vector.tensor_mul(
                        s[:], q_bcast(nu), window_ap(kb, off, u0, nu, PW32))
                    e = ep.tile([128, nu, H, W], bf16, tag=f"e{u0}")
                    nc.scalar.activation(
                        e[:], s[:], mybir.ActivationFunctionType.Exp)
                    if first and u0 == 0:
                        # Remaining bias copies tucked after the first
                        # s-TT emission: the loop starts ~6us earlier
                        # and the copies fill DVE's wait for the first
                        # exp. kbE[4..6] first (needed by v=0's h2).
                        for u_ in range(4, K):
                            nc.vector.tensor_scalar_add(
                                kbE[:, u_], kpE[:, u_:u_ + H],
                                bsb[:, u_:u_ + 1])
                        for u_ in range(K):
                            nc.vector.tensor_scalar_add(
                                kbO[:, u_], kpO[:, u_:u_ + H],
                                bsb[:, u_:u_ + 1])
                    m = mp.tile([128, nu, H, W], bf16, tag=f"m{u0}")
                    nc.vector.tensor_mul(
                        m[:], e[:], window_ap(vp, off, u0, nu, RS))
                    for du in range(nu):
                        u = u0 + du
                        for ch in range(2):
                            rsl = slice(16 * ch, 16 * ch + 16)
                            mm_args.append((ch, rsl, e, m, du, u))

                # den first so the tail's reciprocal (which needs only
                # dps) unblocks before the last num matmuls retire; on
                # the last v, ALL den matmuls precede the num ones,
                # sorted by chunk (per-chunk PSUM tiles keep the deps
                # fine-grained).
                if last:
                    for acc_i, acc_list in ((0, dps), (1, nps)):
                        for want_ch in (0, 1):
                            for ch, rsl, te, tm, du, u in mm_args:
                                if ch != want_ch:
                                    continue
                                t = te if acc_i == 0 else tm
                                nc.tensor.matmul(
                                    acc_list[ch][:], isb[:], t[:, du, rsl, :],
                                    start=False, stop=(u == K - 1),
                                    skip_group_check=True,
                                )
                else:
                    for ch, rsl, te, tm, du, u in mm_args:
                        for kind in ("den", "num"):
                            t = te if kind == "den" else tm
                            acc = dps[ch] if kind == "den" else nps[ch]
                            nc.tensor.matmul(
                                acc[:], isb[:], t[:, du, rsl, :],
                                start=(first and u == 0),
                                stop=False,
                                skip_group_check=True,
                            )

            # ---- normalize and store: both reciprocals first (they
            # overlap the PE's final num matmuls), then multiply + DMA
            # per half on separate queues.
            rden = fin.tile([128, HW], f32)
            outsb = fin.tile([128, HW], f32)
            for ch in (0, 1):
                csl = slice(ch * 512, (ch + 1) * 512)
                nc.vector.reciprocal_approx_fast(
                    out=rden[:, csl], in_=dps[ch][:])
            for ch, eng in ((0, nc.sync), (1, nc.scalar)):
                csl = slice(ch * 512, (ch + 1) * 512)
                nc.vector.tensor_mul(
                    outsb[:, csl], nps[ch][:], rden[:, csl])
                eng.dma_start(out=o_ext[:, csl], in_=outsb[:, csl])

    nc.finalize()
    return nc


def _get_nc():
    if "nc" not in _NC_CACHE:
        _NC_CACHE["nc"] = _build_nc()
    return _NC_CACHE["nc"]


def _prep_in_maps(x, wq, wk, wv, rel_h, rel_w):
    bf = ml_dtypes.bfloat16
    ident = np.eye(128, dtype=bf)
    in_maps = []
    for core in range(N_CORES):
        b, cg = divmod(core, 2)
        xb = np.asarray(x[b], dtype=np.float32)
        if cg == 1:
            xb = xb.transpose(0, 2, 1)
        # [ci_within_chunk, ci_chunk, px] so the SBUF partition dim is
        # outermost in DRAM (batched DMAs copy AP-order to AP-order)
        xb = np.ascontiguousarray(
            xb.reshape(2, 128, HW).transpose(1, 0, 2)).astype(np.float16)
        rows = slice(cg * 128, (cg + 1) * 128)
        wt = np.stack([np.asarray(wq)[rows], np.asarray(wk)[rows],
                       np.asarray(wv)[rows]])          # [3, 128, 256]
        wt = wt.transpose(0, 2, 1).astype(np.float16).reshape(
            3, 2, 128, 128)                            # [wi, ci_chunk, ci, co]
        wt = np.ascontiguousarray(
            wt.transpose(0, 2, 1, 3))                  # [wi, ci, ci_chunk, co]
        bias = np.ascontiguousarray(
            np.asarray(rel_h if cg == 0 else rel_w, dtype=np.float32))
        in_maps.append({"x": xb, "w": wt, "bias": bias, "ident": ident})
    return in_maps


def _assemble(results):
    out = np.empty((B, C, H, W), np.float32)
    for core in range(N_CORES):
        b, cg = divmod(core, 2)
        o = results[core]["out"].reshape(128, H, W)
        if cg == 1:
            o = o.transpose(0, 2, 1)
        out[b, cg * 128:(cg + 1) * 128] = o
    return out


def run(inputs, trace=False):
    """Returns (output, BassKernelResults)."""
    from concourse import bass_utils

    nc = _get_nc()
    in_maps = _prep_in_maps(**inputs)
    last_err = None
    for _attempt in range(3):
        try:
            res = bass_utils.run_bass_kernel_spmd(
                nc, in_maps, core_ids=list(range(N_CORES)), trace=trace)
            return _assemble(res.results), res
        except Exception as err:  # transient NRT device errors
            last_err = err
    raise last_err


def kernel(x, wq, wk, wv, rel_h, rel_w):
    out, _ = run(
        dict(x=x, wq=wq, wk=wk, wv=wv, rel_h=rel_h, rel_w=rel_w),
        trace=bool(os.environ.get("ATTNCONV_TRACE")),
    )
    return out



# revision 33
# speedup vs baseline: 1.0026x; 1.0026x over previous
"""AttentionConv (sparse local attention, 7x7 window, per-channel softmax)
Trainium2 Bass kernel, SPMD across 8 NeuronCores.

Sharding: core i handles batch b = i//2 and channel half cg = i%2
(channels are independent through the whole op: 1x1 convs produce each
output channel from all input channels, and the softmax is per-channel
over the 7x7 window).

The relative-position bias for channels [0,128) is rel_h[u] (window row)
and for channels [128,256) is rel_w[v] (window col). To keep one SPMD
program for all cores, cg=1 cores receive spatially TRANSPOSED x (H<->W)
and their output is transposed back on the host; under that transpose
rel_w becomes a window-row bias, identical in structure to cg=0.

Per-core pipeline (fp16 score path, bf16 value path, f32 accumulate):
  1. PE GEMMs: q,k,v = W @ x in fp16 (negligible rounding vs fp32 for
     this data, half the DMA bytes, 16-bit matmul speed). K=256
     contraction in 2 chunks, N chunks of 512 (one PSUM bank each).
  2. k,v scattered into zero-padded 38x40 planes; each plane stored
     twice (interior at col 3 and col 2) so windowed reads for even AND
     odd window-cols are 4-byte aligned -> DVE 16-bit 2x perf mode.
  3. 7 bias-added copies of each padded k plane (bias for a fixed
     window-row is a per-partition scalar -> tensor_scalar 4x mode).
  4. main loop over window col v (7 iters), u-dim split in (4,3) halves
     for pipelining, diagonal access patterns covering all u at once:
       s = q * k_biased[window]  fp16     (DVE TT, 2x)
       e = exp(s) -> bf16                 (ScalarE ACT, unnormalized --
                                           scores are far inside exp's
                                           f32/bf16 range, so no
                                           max-subtraction pass needed)
       m = e * v[window]  bf16            (DVE TT, 2x)
       num += I @ m ; den += I @ e        (TensorE identity matmuls
                                           accumulating in PSUM f32;
                                           the otherwise-idle PE does
                                           all the j-summation work)
  5. out = num * reciprocal_approx_fast(den); split-queue DMA out.
Engine budget per core: DVE ~62us (bottleneck: 2 multiplies per window
element at 2 elem/cyc/lane), ACT ~46us, PE ~52us, ~85us measured total.
"""

import os

import numpy as np
import ml_dtypes

K = 7
PAD = 3
H = W = 32
HW = H * W
B = 4
C = 256
RS = 40          # padded plane row stride (elements); even => alignment
PR = H + 2 * PAD  # 38 padded rows
PW = PR * RS     # padded plane size per partition
N_CORES = 8

_NC_CACHE = {}


def _build_nc():
    import concourse.bass as bass
    import concourse.tile as tile
    from concourse import mybir, bacc

    bf16 = mybir.dt.bfloat16
    f16 = mybir.dt.float16
    f32 = mybir.dt.float32

    nc = bacc.Bacc(None)
    x_ext = nc.dram_tensor("x", [128, 2, HW], f16, kind="ExternalInput")
    w_ext = nc.dram_tensor("w", [3, 128, 2, 128], f16, kind="ExternalInput")
    b_ext = nc.dram_tensor("bias", [128, K], f32, kind="ExternalInput")
    i_ext = nc.dram_tensor("ident", [128, 128], bf16, kind="ExternalInput")
    # fp16 output: halves the output DMA; |out| <= ~7 so fp16 rounding
    # adds ~3e-4 rel err against the 2e-2 budget. Host casts back.
    o_ext = nc.dram_tensor("out", [128, HW], f16, kind="ExternalOutput")

    with tile.TileContext(nc) as tc:
        with (
            tc.tile_pool(name="consts", bufs=1) as consts,
            tc.tile_pool(name="kv", bufs=1) as kv,
            tc.tile_pool(name="fin", bufs=1) as fin,
            tc.tile_pool(name="psa", bufs=1, space="PSUM") as psa,
            tc.tile_pool(name="gt", bufs=1) as gt,
            tc.tile_pool(name="psg", bufs=4, space="PSUM") as psg,
            tc.tile_pool(name="sp", bufs=3) as sp,
            tc.tile_pool(name="ep", bufs=3) as ep,
            tc.tile_pool(name="mp", bufs=3) as mp,
        ):
            # Batched DMAs: each trigger instruction costs ~650ns of queue
            # time, so few big transfers beat many small ones. k weights
            # and the first pixel-half of x go first (they gate the k GEMM
            # -> scatter -> bias-copy -> main-loop chain).
            xsb = gt.tile([128, 2, HW], f16)
            wsb = gt.tile([128, 3, 2, 128], f16)
            bsb = consts.tile([128, K], f32)
            isb = consts.tile([128, 128], bf16)
            # (GpSimd stays trigger-free: its queue must reach the plane
            # memsets ASAP -- they gate the k scatter. ident goes late:
            # the warm-up matmuls chew on wk instead, and the first real
            # identity matmul only fires ~14us in.)
            nc.sync.dma_start(out=wsb[:, 1], in_=w_ext[1])
            nc.scalar.dma_start(out=xsb[:, :, 0:512], in_=x_ext[:, :, 0:512])
            nc.sync.dma_start(out=xsb[:, :, 512:HW], in_=x_ext[:, :, 512:HW])
            nc.scalar.dma_start(out=wsb[:, 0], in_=w_ext[0])
            nc.sync.dma_start(out=isb[:], in_=i_ext[:])
            nc.scalar.dma_start(out=wsb[:, 2], in_=w_ext[2])
            nc.sync.dma_start(out=bsb[:], in_=b_ext[:])

            # q plane (read via a 7-way broadcast AP in the main loop)
            qsb = kv.tile([128, H, W], f16)
            # padded k/v planes; E holds interior at col 3 (for even v
            # window reads), O at col 2 (odd v reads at offset v-1).
            kpE = gt.tile([128, PR, RS], f16)
            kpO = gt.tile([128, PR, RS], f16)
            vpE = kv.tile([128, PR, RS], bf16)
            vpO = kv.tile([128, PR, RS], bf16)
            # kb planes are stored pre-shifted: kb[u][r] = kp[u+r] + b[u]
            # for r in 0..32 (only rows u..u+31 of a padded plane are
            # ever read for window-row u). Copies shrink 1520->1280
            # elems and the window AP's u-step becomes exactly one
            # plane (PW32).
            PW32 = H * RS
            kbE = kv.tile([128, K, H, RS], f16)
            kbO = kv.tile([128, K, H, RS], f16)

            nc.gpsimd.memset(kpE[:], 0.0)
            nc.gpsimd.memset(kpO[:], 0.0)
            nc.gpsimd.memset(vpE[:], 0.0)
            nc.gpsimd.memset(vpO[:], 0.0)

            # per-chunk PSUM accumulators (finer deps => earlier tail)
            nps0 = psa.tile([128, 512], f32)
            nps1 = psa.tile([128, 512], f32)
            dps0 = psa.tile([128, 512], f32)
            dps1 = psa.tile([128, 512], f32)
            nps = [nps0, nps1]
            dps = [dps0, dps1]

            # PE pipeline/HAM warm-up: dummy matmuls into nps0, whose
            # content is discarded when the first start=True accumulation
            # clears has_written. They chew on the k weights (first DMA
            # to land) so they don't wait for the ident transfer.
            for _ in range(3):
                nc.tensor.matmul(nps0[:, 0:128], wsb[:, 1, 0, :],
                                 wsb[:, 1, 0, :],
                                 start=True, stop=True, skip_group_check=True)

            # ---- GEMMs: wi 0=q, 1=k, 2=v; N chunks of 512 px (16 rows)
            # k+q matmuls first (4 PSUM bufs hold k0,k1,q0,q1), with the
            # ACT scatter order kpE -> q -> kpO so the chain that gates
            # the first main-loop TT (kpE scatter -> kbE copies on DVE)
            # is as early as possible. v GEMMs reuse the k PSUM bufs
            # after the kpO scatter releases them.
            def gemm(wi, ch):
                ps = psg.tile([128, 16, 32], f32, tag="ps", name="ps")
                for ci in range(2):
                    nc.tensor.matmul(
                        ps[:],
                        wsb[:, wi, ci, :],
                        xsb[:, ci, ch * 512:(ch + 1) * 512],
                        start=(ci == 0),
                        stop=(ci == 1),
                    )
                return ps

            ps_k = [gemm(1, ch) for ch in range(2)]
            ps_q = [gemm(0, ch) for ch in range(2)]
            # biased k copies: kb*[u] = kp* + bias[:, u] (per-partition).
            # All on DVE (4x tensor_scalar mode; GpSimd's tensor_scalar
            # ucode measures ~22us/plane regardless of AP shape, and ACT
            # has no slack once the exp stream starts). The gate for the
            # first TT -- kbE[0..3] -- is split by plane row-halves so
            # the ch0-row copies overlap the k-ch1 GEMM+scatter;
            # kbE[4..6] and all of kbO are emitted inside v=0's body
            # where they fill DVE's wait for the first exp.
            nc.scalar.copy(kpE[:, PAD:PAD + 16, 3:35], ps_k[0][:])
            for u in range(4):
                b0 = PAD + 16 - u  # shifted-plane row where ch1 data starts
                nc.vector.tensor_scalar_add(
                    kbE[:, u, 0:b0], kpE[:, u:u + b0], bsb[:, u:u + 1])
            nc.scalar.copy(kpE[:, PAD + 16:PAD + 32, 3:35], ps_k[1][:])
            for ch in range(2):
                nc.scalar.copy(qsb[:, 16 * ch:16 * ch + 16, :], ps_q[ch][:])
            for u in range(4):
                b0 = PAD + 16 - u
                nc.vector.tensor_scalar_add(
                    kbE[:, u, b0:H], kpE[:, u + b0:u + H],
                    bsb[:, u:u + 1])
            for ch in range(2):
                r0 = PAD + 16 * ch
                nc.scalar.copy(kpO[:, r0:r0 + 16, 2:34], ps_k[ch][:])
            for ch in range(2):
                r0 = PAD + 16 * ch
                ps = gemm(2, ch)
                nc.scalar.copy(vpE[:, r0:r0 + 16, 3:35], ps[:])
                nc.scalar.copy(vpO[:, r0:r0 + 16, 2:34], ps[:])

            def window_ap(t, base_off, u0, nu, u_step):
                full = t[:]
                return bass.AP(
                    tensor=full.tensor,
                    offset=full.offset + base_off + u0 * u_step,
                    ap=[full.ap[0], [u_step, nu], [RS, H], [1, W]],
                )

            def q_bcast(nu):
                full = qsb[:]
                return bass.AP(
                    tensor=full.tensor,
                    offset=full.offset,
                    ap=[full.ap[0], [0, nu], [W, H], [1, W]],
                )

            # u-dim halves for finer DVE->ACT->PE pipelining. Measured
            # optimum: full-width ops 89.8us, quarters 87.2us, (3,4)
            # order 88.2us, this (4,3) split 85.4us. GpSimd offloads of
            # any slab regress (its in-loop TT is 3-5x slower than DVE).
            HALVES = ((0, 4), (4, 3))

            # ---- main loop over window col v (all elementwise on DVE;
            # GpSimd in-loop offload measured 4.2ns/elem under SBUF
            # contention + 1.7us drains and cannot hold the ~6.5us/v
            # cadence -- it lagged the whole loop to 133us)
            for v in range(K):
                par = v & 1
                kb = kbO if par else kbE
                vp = vpO if par else vpE
                off = v - par  # even

                first = v == 0
                last = v == K - 1
                mm_args = []
                for u0, nu in HALVES:
                    s = sp.tile([128, nu, H, W], f16, tag=f"s{u0}")
                    nc.vector.tensor_mul(
                        s[:], q_bcast(nu), window_ap(kb, off, u0, nu, PW32))
                    e = ep.tile([128, nu, H, W], bf16, tag=f"e{u0}")
                    nc.scalar.activation(
                        e[:], s[:], mybir.ActivationFunctionType.Exp)
                    if first and u0 == 0:
                        # Remaining bias copies tucked after the first
                        # s-TT emission: the loop starts ~6us earlier
                        # and the copies fill DVE's wait for the first
                        # exp. kbE[4..6] first (needed by v=0's h2).
                        for u_ in range(4, K):
                            nc.vector.tensor_scalar_add(
                                kbE[:, u_], kpE[:, u_:u_ + H],
                                bsb[:, u_:u_ + 1])
                        for u_ in range(K):
                            nc.vector.tensor_scalar_add(
                                kbO[:, u_], kpO[:, u_:u_ + H],
                                bsb[:, u_:u_ + 1])
                    m = mp.tile([128, nu, H, W], bf16, tag=f"m{u0}")
                    nc.vector.tensor_mul(
                        m[:], e[:], window_ap(vp, off, u0, nu, RS))
                    for du in range(nu):
                        u = u0 + du
                        for ch in range(2):
                            rsl = slice(16 * ch, 16 * ch + 16)
                            mm_args.append((ch, rsl, e, m, du, u))

                # den first so the tail's reciprocal (which needs only
                # dps) unblocks before the last num matmuls retire; on
                # the last v, ALL den matmuls precede the num ones,
                # sorted by chunk (per-chunk PSUM tiles keep the deps
                # fine-grained).
                if last:
                    for acc_i, acc_list in ((0, dps), (1, nps)):
                        for want_ch in (0, 1):
                            for ch, rsl, te, tm, du, u in mm_args:
                                if ch != want_ch:
                                    continue
                                t = te if acc_i == 0 else tm
                                nc.tensor.matmul(
                                    acc_list[ch][:], isb[:], t[:, du, rsl, :],
                                    start=False, stop=(u == K - 1),
                                    skip_group_check=True,
                                )
                else:
                    for ch, rsl, te, tm, du, u in mm_args:
                        for kind in ("den", "num"):
                            t = te if kind == "den" else tm
                            acc = dps[ch] if kind == "den" else nps[ch]
                            nc.tensor.matmul(
                                acc[:], isb[:], t[:, du, rsl, :],
                                start=(first and u == 0),
                                stop=False,
                                skip_group_check=True,
                            )

            # ---- normalize and store: both reciprocals first (they
            # overlap the PE's final num matmuls), then multiply + DMA
            # per half on separate queues.
            rden = fin.tile([128, HW], f32)
            outsb = fin.tile([128, HW], f16)
            for ch in (0, 1):
                csl = slice(ch * 512, (ch + 1) * 512)
                nc.vector.reciprocal_approx_fast(
                    out=rden[:, csl], in_=dps[ch][:])
            for ch, eng in ((0, nc.sync), (1, nc.scalar)):
                csl = slice(ch * 512, (ch + 1) * 512)
                nc.vector.tensor_mul(
                    outsb[:, csl], nps[ch][:], rden[:, csl])
                eng.dma_start(out=o_ext[:, csl], in_=outsb[:, csl])

    nc.finalize()
    return nc


def _get_nc():
    if "nc" not in _NC_CACHE:
        _NC_CACHE["nc"] = _build_nc()
    return _NC_CACHE["nc"]


def _prep_in_maps(x, wq, wk, wv, rel_h, rel_w):
    bf = ml_dtypes.bfloat16
    ident = np.eye(128, dtype=bf)
    in_maps = []
    for core in range(N_CORES):
        b, cg = divmod(core, 2)
        xb = np.asarray(x[b], dtype=np.float32)
        if cg == 1:
            xb = xb.transpose(0, 2, 1)
        # [ci_within_chunk, ci_chunk, px] so the SBUF partition dim is
        # outermost in DRAM (batched DMAs copy AP-order to AP-order)
        xb = np.ascontiguousarray(
            xb.reshape(2, 128, HW).transpose(1, 0, 2)).astype(np.float16)
        rows = slice(cg * 128, (cg + 1) * 128)
        wt = np.stack([np.asarray(wq)[rows], np.asarray(wk)[rows],
                       np.asarray(wv)[rows]])          # [3, 128, 256]
        wt = wt.transpose(0, 2, 1).astype(np.float16).reshape(
            3, 2, 128, 128)                            # [wi, ci_chunk, ci, co]
        wt = np.ascontiguousarray(
            wt.transpose(0, 2, 1, 3))                  # [wi, ci, ci_chunk, co]
        bias = np.ascontiguousarray(
            np.asarray(rel_h if cg == 0 else rel_w, dtype=np.float32))
        in_maps.append({"x": xb, "w": wt, "bias": bias, "ident": ident})
    return in_maps


def _assemble(results):
    out = np.empty((B, C, H, W), np.float32)
    for core in range(N_CORES):
        b, cg = divmod(core, 2)
        o = results[core]["out"].reshape(128, H, W)
        if cg == 1:
            o = o.transpose(0, 2, 1)
        out[b, cg * 128:(cg + 1) * 128] = o
    return out


def run(inputs, trace=False):
    """Returns (output, BassKernelResults)."""
    from concourse import bass_utils

    nc = _get_nc()
    in_maps = _prep_in_maps(**inputs)
    last_err = None
    for _attempt in range(3):
        try:
            res = bass_utils.run_bass_kernel_spmd(
                nc, in_maps, core_ids=list(range(N_CORES)), trace=trace)
            return _assemble(res.results), res
        except Exception as err:  # transient NRT device errors
            last_err = err
    raise last_err


def kernel(x, wq, wk, wv, rel_h, rel_w):
    out, _ = run(
        dict(x=x, wq=wq, wk=wk, wv=wv, rel_h=rel_h, rel_w=rel_w),
        trace=bool(os.environ.get("ATTNCONV_TRACE")),
    )
    return out



# revision 34
# speedup vs baseline: 1.1886x; 1.1855x over previous
"""AttentionConv (sparse local attention, 7x7 window, per-channel softmax)
Trainium2 Bass kernel, SPMD across 8 NeuronCores.

Sharding: core i handles batch b = i//2 and channel half cg = i%2
(channels are independent through the whole op: 1x1 convs produce each
output channel from all input channels, and the softmax is per-channel
over the 7x7 window).

The relative-position bias for channels [0,128) is rel_h[u] (window row)
and for channels [128,256) is rel_w[v] (window col). To keep one SPMD
program for all cores, cg=1 cores receive spatially TRANSPOSED x (H<->W)
and their output is transposed back on the host; under that transpose
rel_w becomes a window-row bias, identical in structure to cg=0.

Per-core pipeline (fp16 score path, bf16 value path, f32 accumulate):
  1. Batched DMAs (triggers cost ~650ns of queue time each): k weights
     and the first pixel-half of x first; ident late (only needed by
     the first identity matmul ~14us in; PE warm-up chews on wk).
  2. PE GEMMs: q,k,v = W @ x in fp16 (negligible rounding vs fp32 for
     this data, half the DMA bytes, 16-bit matmul speed). K=256
     contraction in 2 chunks, N chunks of 512 (one PSUM bank each).
     k+q GEMMs first; ACT scatter order kpE -> q -> kpO -> v so the
     chain gating the first main-loop TT is earliest.
  3. k,v scattered into zero-padded planes (ACT); each plane stored
     twice (interior at col 3 and col 2) so windowed reads for even AND
     odd window-cols are 4-byte aligned -> DVE 16-bit 2x perf mode.
  4. 7 bias-added copies of each k plane, PRE-SHIFTED to 32 rows:
     kb[u][r] = kp[u+r] + b[u] (only rows u..u+31 are ever read for
     window-row u), tensor_scalar 4x mode, 1280 elems each. kbE[0..3]
     (the first-TT gate) split by GEMM row-halves to overlap the k-ch1
     GEMM; the other 10 copies are emitted inside v=0's body where
     they fill DVE's wait for the first exp.
  5. main loop over window col v (7 iters), u-dim split in (4,3) halves
     for pipelining, diagonal access patterns covering all u at once:
       s = q * k_biased[window]  fp16     (DVE TT, 2x)
       e = exp(s) -> bf16                 (ScalarE ACT, unnormalized --
                                           scores are far inside exp's
                                           f32/bf16 range, so no
                                           max-subtraction pass needed)
       m = e * v[window]  bf16            (DVE TT, 2x)
       num += I @ m ; den += I @ e        (TensorE identity matmuls
                                           accumulating in per-chunk
                                           PSUM f32 tiles; the
                                           otherwise-idle PE does all
                                           the j-summation work)
  6. Tail: last-v den matmuls chunk-sorted ahead of num ones so the
     reciprocals overlap the final matmuls; out = num * recip(den)
     written as fp16 (halves the out DMA; +3e-4 rel err); split-queue
     DMA out.
Steady state is DVE-bound (~7.7us/v vs ACT ~6.5us/v): DVE runs gapless
from the first TT to the tail. Measured 83.8us (fast clock state) /
97.3us (slow state; chip oscillates, everything scales ~1.16x).
Rejected via measurement: GpSimd tensor_scalar (~22us/plane any AP
shape), GpSimd in-loop TT offload (4.2ns/elem under SBUF contention +
1.7us drains, lags the cadence), ACT Identity-bias copies (1.36us vs
DVE's 0.39us), stride-0-out du-merged matmuls (>512-col ISA reject),
scalar_tensor_tensor bias fusion (1x perf mode only).
"""

import os

import numpy as np
import ml_dtypes

K = 7
PAD = 3
H = W = 32
HW = H * W
B = 4
C = 256
RS = 40          # padded plane row stride (elements); even => alignment
PR = H + 2 * PAD  # 38 padded rows
PW = PR * RS     # padded plane size per partition
N_CORES = 8

_NC_CACHE = {}


def _build_nc():
    import concourse.bass as bass
    import concourse.tile as tile
    from concourse import mybir, bacc

    bf16 = mybir.dt.bfloat16
    f16 = mybir.dt.float16
    f32 = mybir.dt.float32

    nc = bacc.Bacc(None)
    x_ext = nc.dram_tensor("x", [128, 2, HW], f16, kind="ExternalInput")
    w_ext = nc.dram_tensor("w", [3, 128, 2, 128], f16, kind="ExternalInput")
    b_ext = nc.dram_tensor("bias", [128, K], f32, kind="ExternalInput")
    i_ext = nc.dram_tensor("ident", [128, 128], bf16, kind="ExternalInput")
    # fp16 output: halves the output DMA; |out| <= ~7 so fp16 rounding
    # adds ~3e-4 rel err against the 2e-2 budget. Host casts back.
    o_ext = nc.dram_tensor("out", [128, HW], f16, kind="ExternalOutput")

    with tile.TileContext(nc) as tc:
        with (
            tc.tile_pool(name="consts", bufs=1) as consts,
            tc.tile_pool(name="kv", bufs=1) as kv,
            tc.tile_pool(name="fin", bufs=1) as fin,
            tc.tile_pool(name="psa", bufs=1, space="PSUM") as psa,
            tc.tile_pool(name="gt", bufs=1) as gt,
            tc.tile_pool(name="psg", bufs=4, space="PSUM") as psg,
            tc.tile_pool(name="sp", bufs=3) as sp,
            tc.tile_pool(name="ep", bufs=3) as ep,
            tc.tile_pool(name="mp", bufs=3) as mp,
        ):
            # Batched DMAs: each trigger instruction costs ~650ns of queue
            # time, so few big transfers beat many small ones. k weights
            # and the first pixel-half of x go first (they gate the k GEMM
            # -> scatter -> bias-copy -> main-loop chain).
            xsb = gt.tile([128, 2, HW], f16)
            wsb = gt.tile([128, 3, 2, 128], f16)
            bsb = consts.tile([128, K], f32)
            isb = consts.tile([128, 128], bf16)
            # (GpSimd stays trigger-free: its queue must reach the plane
            # memsets ASAP -- they gate the k scatter. ident goes late:
            # the warm-up matmuls chew on wk instead, and the first real
            # identity matmul only fires ~14us in.)
            nc.sync.dma_start(out=wsb[:, 1], in_=w_ext[1])
            nc.scalar.dma_start(out=xsb[:, :, 0:512], in_=x_ext[:, :, 0:512])
            nc.sync.dma_start(out=xsb[:, :, 512:HW], in_=x_ext[:, :, 512:HW])
            nc.scalar.dma_start(out=wsb[:, 0], in_=w_ext[0])
            nc.sync.dma_start(out=isb[:], in_=i_ext[:])
            nc.scalar.dma_start(out=wsb[:, 2], in_=w_ext[2])
            nc.sync.dma_start(out=bsb[:], in_=b_ext[:])

            # q plane (read via a 7-way broadcast AP in the main loop)
            qsb = kv.tile([128, H, W], f16)
            # padded k/v planes; E holds interior at col 3 (for even v
            # window reads), O at col 2 (odd v reads at offset v-1).
            kpE = gt.tile([128, PR, RS], f16)
            kpO = gt.tile([128, PR, RS], f16)
            vpE = kv.tile([128, PR, RS], bf16)
            vpO = kv.tile([128, PR, RS], bf16)
            # kb planes are stored pre-shifted: kb[u][r] = kp[u+r] + b[u]
            # for r in 0..32 (only rows u..u+31 of a padded plane are
            # ever read for window-row u). Copies shrink 1520->1280
            # elems and the window AP's u-step becomes exactly one
            # plane (PW32).
            PW32 = H * RS
            kbE = kv.tile([128, K, H, RS], f16)
            kbO = kv.tile([128, K, H, RS], f16)

            nc.gpsimd.memset(kpE[:], 0.0)
            nc.gpsimd.memset(kpO[:], 0.0)
            nc.gpsimd.memset(vpE[:], 0.0)
            nc.gpsimd.memset(vpO[:], 0.0)

            # per-chunk PSUM accumulators (finer deps => earlier tail)
            nps0 = psa.tile([128, 512], f32)
            nps1 = psa.tile([128, 512], f32)
            dps0 = psa.tile([128, 512], f32)
            dps1 = psa.tile([128, 512], f32)
            nps = [nps0, nps1]
            dps = [dps0, dps1]

            # PE pipeline/HAM warm-up: dummy matmuls into nps0, whose
            # content is discarded when the first start=True accumulation
            # clears has_written. They chew on the k weights (first DMA
            # to land) so they don't wait for the ident transfer.
            for _ in range(3):
                nc.tensor.matmul(nps0[:, 0:128], wsb[:, 1, 0, :],
                                 wsb[:, 1, 0, :],
                                 start=True, stop=True, skip_group_check=True)

            # ---- GEMMs: wi 0=q, 1=k, 2=v; N chunks of 512 px (16 rows)
            # k+q matmuls first (4 PSUM bufs hold k0,k1,q0,q1), with the
            # ACT scatter order kpE -> q -> kpO so the chain that gates
            # the first main-loop TT (kpE scatter -> kbE copies on DVE)
            # is as early as possible. v GEMMs reuse the k PSUM bufs
            # after the kpO scatter releases them.
            def gemm(wi, ch):
                ps = psg.tile([128, 16, 32], f32, tag="ps", name="ps")
                for ci in range(2):
                    nc.tensor.matmul(
                        ps[:],
                        wsb[:, wi, ci, :],
                        xsb[:, ci, ch * 512:(ch + 1) * 512],
                        start=(ci == 0),
                        stop=(ci == 1),
                    )
                return ps

            ps_k = [gemm(1, ch) for ch in range(2)]
            ps_q = [gemm(0, ch) for ch in range(2)]
            # biased k copies: kb*[u] = kp* + bias[:, u] (per-partition).
            # All on DVE (4x tensor_scalar mode; GpSimd's tensor_scalar
            # ucode measures ~22us/plane regardless of AP shape, and ACT
            # has no slack once the exp stream starts). The gate for the
            # first TT -- kbE[0..3] -- is split by plane row-halves so
            # the ch0-row copies overlap the k-ch1 GEMM+scatter;
            # kbE[4..6] and all of kbO are emitted inside v=0's body
            # where they fill DVE's wait for the first exp.
            nc.scalar.copy(kpE[:, PAD:PAD + 16, 3:35], ps_k[0][:])
            for u in range(4):
                b0 = PAD + 16 - u  # shifted-plane row where ch1 data starts
                nc.vector.tensor_scalar_add(
                    kbE[:, u, 0:b0], kpE[:, u:u + b0], bsb[:, u:u + 1])
            nc.scalar.copy(kpE[:, PAD + 16:PAD + 32, 3:35], ps_k[1][:])
            for ch in range(2):
                nc.scalar.copy(qsb[:, 16 * ch:16 * ch + 16, :], ps_q[ch][:])
            for u in range(4):
                b0 = PAD + 16 - u
                nc.vector.tensor_scalar_add(
                    kbE[:, u, b0:H], kpE[:, u + b0:u + H],
                    bsb[:, u:u + 1])
            for ch in range(2):
                r0 = PAD + 16 * ch
                nc.scalar.copy(kpO[:, r0:r0 + 16, 2:34], ps_k[ch][:])
            for ch in range(2):
                r0 = PAD + 16 * ch
                ps = gemm(2, ch)
                nc.scalar.copy(vpE[:, r0:r0 + 16, 3:35], ps[:])
                nc.scalar.copy(vpO[:, r0:r0 + 16, 2:34], ps[:])

            def window_ap(t, base_off, u0, nu, u_step):
                full = t[:]
                return bass.AP(
                    tensor=full.tensor,
                    offset=full.offset + base_off + u0 * u_step,
                    ap=[full.ap[0], [u_step, nu], [RS, H], [1, W]],
                )

            def q_bcast(nu):
                full = qsb[:]
                return bass.AP(
                    tensor=full.tensor,
                    offset=full.offset,
                    ap=[full.ap[0], [0, nu], [W, H], [1, W]],
                )

            # u-dim halves for finer DVE->ACT->PE pipelining. Measured
            # optimum: full-width ops 89.8us, quarters 87.2us, (3,4)
            # order 88.2us, this (4,3) split 85.4us. GpSimd offloads of
            # any slab regress (its in-loop TT is 3-5x slower than DVE).
            HALVES = ((0, 4), (4, 3))

            # ---- main loop over window col v (all elementwise on DVE;
            # GpSimd in-loop offload measured 4.2ns/elem under SBUF
            # contention + 1.7us drains and cannot hold the ~6.5us/v
            # cadence -- it lagged the whole loop to 133us)
            for v in range(K):
                par = v & 1
                kb = kbO if par else kbE
                vp = vpO if par else vpE
                off = v - par  # even

                first = v == 0
                last = v == K - 1
                mm_args = []
                for u0, nu in HALVES:
                    s = sp.tile([128, nu, H, W], f16, tag=f"s{u0}")
                    nc.vector.tensor_mul(
                        s[:], q_bcast(nu), window_ap(kb, off, u0, nu, PW32))
                    e = ep.tile([128, nu, H, W], bf16, tag=f"e{u0}")
                    nc.scalar.activation(
                        e[:], s[:], mybir.ActivationFunctionType.Exp)
                    if first and u0 == 0:
                        # Remaining bias copies tucked after the first
                        # s-TT emission: the loop starts ~6us earlier
                        # and the copies fill DVE's wait for the first
                        # exp. kbE[4..6] first (needed by v=0's h2).
                        for u_ in range(4, K):
                            nc.vector.tensor_scalar_add(
                                kbE[:, u_], kpE[:, u_:u_ + H],
                                bsb[:, u_:u_ + 1])
                        for u_ in range(K):
                            nc.vector.tensor_scalar_add(
                                kbO[:, u_], kpO[:, u_:u_ + H],
                                bsb[:, u_:u_ + 1])
                    m = mp.tile([128, nu, H, W], bf16, tag=f"m{u0}")
                    nc.vector.tensor_mul(
                        m[:], e[:], window_ap(vp, off, u0, nu, RS))
                    for du in range(nu):
                        u = u0 + du
                        for ch in range(2):
                            rsl = slice(16 * ch, 16 * ch + 16)
                            mm_args.append((ch, rsl, e, m, du, u))

                # den first so the tail's reciprocal (which needs only
                # dps) unblocks before the last num matmuls retire; on
                # the last v, ALL den matmuls precede the num ones,
                # sorted by chunk (per-chunk PSUM tiles keep the deps
                # fine-grained).
                if last:
                    for acc_i, acc_list in ((0, dps), (1, nps)):
                        for want_ch in (0, 1):
                            for ch, rsl, te, tm, du, u in mm_args:
                                if ch != want_ch:
                                    continue
                                t = te if acc_i == 0 else tm
                                nc.tensor.matmul(
                                    acc_list[ch][:], isb[:], t[:, du, rsl, :],
                                    start=False, stop=(u == K - 1),
                                    skip_group_check=True,
                                )
                else:
                    for ch, rsl, te, tm, du, u in mm_args:
                        for kind in ("den", "num"):
                            t = te if kind == "den" else tm
                            acc = dps[ch] if kind == "den" else nps[ch]
                            nc.tensor.matmul(
                                acc[:], isb[:], t[:, du, rsl, :],
                                start=(first and u == 0),
                                stop=False,
                                skip_group_check=True,
                            )

            # ---- normalize and store: both reciprocals first (they
            # overlap the PE's final num matmuls), then multiply + DMA
            # per half on separate queues.
            rden = fin.tile([128, HW], f32)
            outsb = fin.tile([128, HW], f16)
            for ch in (0, 1):
                csl = slice(ch * 512, (ch + 1) * 512)
                nc.vector.reciprocal_approx_fast(
                    out=rden[:, csl], in_=dps[ch][:])
            for ch, eng in ((0, nc.sync), (1, nc.scalar)):
                csl = slice(ch * 512, (ch + 1) * 512)
                nc.vector.tensor_mul(
                    outsb[:, csl], nps[ch][:], rden[:, csl])
                eng.dma_start(out=o_ext[:, csl], in_=outsb[:, csl])

    nc.finalize()
    return nc


def _get_nc():
    if "nc" not in _NC_CACHE:
        _NC_CACHE["nc"] = _build_nc()
    return _NC_CACHE["nc"]


def _prep_in_maps(x, wq, wk, wv, rel_h, rel_w):
    bf = ml_dtypes.bfloat16
    ident = np.eye(128, dtype=bf)
    in_maps = []
    for core in range(N_CORES):
        b, cg = divmod(core, 2)
        xb = np.asarray(x[b], dtype=np.float32)
        if cg == 1:
            xb = xb.transpose(0, 2, 1)
        # [ci_within_chunk, ci_chunk, px] so the SBUF partition dim is
        # outermost in DRAM (batched DMAs copy AP-order to AP-order)
        xb = np.ascontiguousarray(
            xb.reshape(2, 128, HW).transpose(1, 0, 2)).astype(np.float16)
        rows = slice(cg * 128, (cg + 1) * 128)
        wt = np.stack([np.asarray(wq)[rows], np.asarray(wk)[rows],
                       np.asarray(wv)[rows]])          # [3, 128, 256]
        wt = wt.transpose(0, 2, 1).astype(np.float16).reshape(
            3, 2, 128, 128)                            # [wi, ci_chunk, ci, co]
        wt = np.ascontiguousarray(
            wt.transpose(0, 2, 1, 3))                  # [wi, ci, ci_chunk, co]
        bias = np.ascontiguousarray(
            np.asarray(rel_h if cg == 0 else rel_w, dtype=np.float32))
        in_maps.append({"x": xb, "w": wt, "bias": bias, "ident": ident})
    return in_maps


def _assemble(results):
    out = np.empty((B, C, H, W), np.float32)
    for core in range(N_CORES):
        b, cg = divmod(core, 2)
        o = results[core]["out"].reshape(128, H, W)
        if cg == 1:
            o = o.transpose(0, 2, 1)
        out[b, cg * 128:(cg + 1) * 128] = o
    return out


def run(inputs, trace=False):
    """Returns (output, BassKernelResults)."""
    from concourse import bass_utils

    nc = _get_nc()
    in_maps = _prep_in_maps(**inputs)
    last_err = None
    for _attempt in range(3):
        try:
            res = bass_utils.run_bass_kernel_spmd(
                nc, in_maps, core_ids=list(range(N_CORES)), trace=trace)
            return _assemble(res.results), res
        except Exception as err:  # transient NRT device errors
            last_err = err
    raise last_err


def kernel(x, wq, wk, wv, rel_h, rel_w):
    out, _ = run(
        dict(x=x, wq=wq, wk=wk, wv=wv, rel_h=rel_h, rel_w=rel_w),
        trace=bool(os.environ.get("ATTNCONV_TRACE")),
    )
    return out

